# revision 1
# baseline (speedup 1.0000x reference)
"""Grouped-Query Attention on 8 Trainium2 NeuronCores.

Sharding: TP-4 over KV groups x DP-2 over batch.
Core c handles batch b = c // 4, group g = c % 4 (4 query heads, 1 KV group).
Each core computes q/k/v projections for its heads, causal attention, and a
partial O-projection (its 512 input columns of Wo); the host sums the 4 TP
partials per batch and adds bo.

All matmuls run in bf16 with fp32 PSUM accumulation.  Layout is fully
"transposed" on device so no on-chip transposes of activations are needed:
  qT, kT: [d=128 partitions, t]        (proj computed as W^T @ x^T)
  S^T tiles: [tk=128, <=512 q] = kT_blk.T @ qT, exact-causal column ranges
  E = exp(S^T * scale); the 128x128 diagonal triangle is masked on GPSIMD
  row-sums of softmax = ones128^T @ E (PE) -- fused sum+broadcast
  attn^T [d, tq] = V^T @ E accumulated over tk blocks (V natural [tk, d])
  out [tq, e] partial = attn^T.T @ Wo_rows accumulated over the 4 heads

Scheduling notes:
  - k/v projections are interleaved per contraction chunk (v lagging) so PE
    consumption of the xt DMA stream never outruns HBM bandwidth, and k's
    PSUM drains overlap v's tail.
  - big weights (wq per-head, wo) are queued on the SP DGE *after* the xt
    chunks so they don't delay the chunk cadence the k-projection feeds on.
  - V is re-laid out [tk, d] by SBUF-to-SBUF DMA transposes (XBAR), off PE.
  - q projections run on the O-projection PSUM ring inside the attention
    pool scope, so no pool-close barrier separates phase 1 from attention.
  - attention is software-pipelined two slots deep: slot i's S/exp stream
    interleaves with slot i-2's rowsum+AV matmuls, so PE never waits on ACT.
  - PSUM banks: s2 x3, rowsum x1, AV x2, O-proj x2 = 8 exactly.
"""

import numpy as np
import ml_dtypes

EMBED = 2048
T = 2048
D = 128           # head dim
NQH = 16          # query heads
NG = 4            # kv groups
HPG = NQH // NG   # query heads per group = 4
NCORES = 8
ECH = EMBED // 128   # 16 contraction chunks
TC = T // 512        # 4 q-chunks of 512
TT = T // 128        # 16 t-tiles of 128
SCALE = 1.0 / float(np.sqrt(D))

_PROG = {}


def build_program():
    if "nc" in _PROG:
        return _PROG["nc"]

    from contextlib import ExitStack
    import concourse.mybir as mybir
    from concourse import bacc, tile
    from concourse.masks import make_identity

    # The Tile legalizer emits one Ldweights per Matmult even when consecutive
    # matmuls reuse the same stationary operand; dropping the repeats saves PE
    # sequencer time.  Ldweights whose stationary is the ones128 constant are
    # exempt: the compiler later moves the following matmul's waits onto its
    # Ldweights, and dropping one of those desynchronizes the broadcast reads.
    if not getattr(tile.tile_legalize, "_ldw_dedup", False):
        _orig_legalize = tile.tile_legalize

        def _dedup_legalize(ordered, nc_):
            ordered = _orig_legalize(ordered, nc_)
            for bb, insts in ordered.items():
                out = []
                state = None
                for inst in insts:
                    tn = type(inst).__name__
                    if tn == "InstLdweights":
                        key = (
                            str(inst.ins[0]),
                            str(getattr(inst, "is_transpose", None)),
                            str(getattr(inst, "tile_position", None)),
                            str(getattr(inst, "perf_mode", None)),
                        )
                        if key == state and "ones128" not in key[0]:
                            continue
                        state = key
                    out.append(inst)
                ordered[bb] = out
            return ordered

        _dedup_legalize._ldw_dedup = True
        tile.tile_legalize = _dedup_legalize

    dt = mybir.dt
    BF = dt.bfloat16
    F32 = dt.float32
    AF = mybir.ActivationFunctionType

    nc = bacc.Bacc("TRN2", target_bir_lowering=False, debug=False)

    xt_d = nc.dram_tensor("xt", [ECH, 128, T], BF, kind="ExternalInput")
    wq_d = nc.dram_tensor("wq", [128, HPG, ECH * D], BF, kind="ExternalInput")
    wk_d = nc.dram_tensor("wk", [128, ECH * D], BF, kind="ExternalInput")
    wv_d = nc.dram_tensor("wv", [128, ECH * D], BF, kind="ExternalInput")
    wo_d = nc.dram_tensor("wo", [128, HPG * EMBED], BF, kind="ExternalInput")
    tril_d = nc.dram_tensor("tril", [128, 128], BF, kind="ExternalInput")
    bq_d = nc.dram_tensor("bq", [128, HPG], F32, kind="ExternalInput")
    bk_d = nc.dram_tensor("bk", [128, 1], F32, kind="ExternalInput")
    bv_d = nc.dram_tensor("bv", [128, 1], F32, kind="ExternalInput")
    out_d = nc.dram_tensor("out", [T, EMBED], BF, kind="ExternalOutput")

    with tile.TileContext(nc) as tc, ExitStack() as ctx:
        pers = ctx.enter_context(tc.tile_pool(name="pers", bufs=1))

        wq_sb = pers.tile([128, HPG, ECH * D], BF)
        wk_sb = pers.tile([128, ECH * D], BF)
        wv_sb = pers.tile([128, ECH * D], BF)
        wo_sb = pers.tile([128, HPG * EMBED], BF)
        tril_sb = pers.tile([128, 128], BF)
        bq_sb = pers.tile([128, HPG], F32)
        bk_sb = pers.tile([128, 1], F32)
        bv_sb = pers.tile([128, 1], F32)
        qT_sb = pers.tile([128, HPG, T], BF)
        kT_sb = pers.tile([128, T], BF)
        vT_sb = pers.tile([128, T], BF)
        v_sb = pers.tile([128, TT, D], BF)
        ones128 = pers.tile([128, 128], BF)
        ident = pers.tile([128, 128], BF)

        nc.gpsimd.memset(ones128[:], 1.0)
        make_identity(nc, ident[:])

        # ACT DGE: small early constants.
        nc.scalar.dma_start(bk_sb[:], bk_d[:])
        nc.scalar.dma_start(bv_sb[:], bv_d[:])
        nc.scalar.dma_start(bq_sb[:], bq_d[:])

        def drain_proj(ps, sl, j, dst, par):
            """one 512-col psum chunk -> sbuf bf16 + bias, ACT/DVE by parity."""
            if j == 0:
                b_ap = bk_sb[:]
            elif j == 1:
                b_ap = bv_sb[:]
            else:
                b_ap = bq_sb[:, j - 2:j - 1]
            if par % 2 == 0:
                nc.scalar.activation(dst, ps[:, sl], AF.Identity, bias=b_ap)
            else:
                nc.vector.tensor_scalar_add(dst, ps[:, sl], b_ap)

        with tc.tile_pool(name="xtp", bufs=1) as xtp:
            xt_sb = xtp.tile([128, ECH, T], BF)

            # SP DGE issue order == DMA-engine service order (FIFO).  The xt
            # chunk cadence feeds the k-projection exactly at HBM rate; small
            # weight pieces are slotted where the consumption schedule has
            # slack, big wo/tril ride behind everything time-critical.
            def emit_xt(ec):
                nc.sync.dma_start(xt_sb[:, ec, :], xt_d[ec])

            nc.sync.dma_start(wk_sb[:, :ECH * D // 2], wk_d[:, :ECH * D // 2])
            emit_xt(0)
            emit_xt(1)
            emit_xt(2)
            nc.sync.dma_start(wv_sb[:], wv_d[:])
            for ec in range(3, 7):
                emit_xt(ec)
            nc.sync.dma_start(wk_sb[:, ECH * D // 2:], wk_d[:, ECH * D // 2:])
            for ec in range(7, 13):
                emit_xt(ec)
            nc.sync.dma_start(wq_sb[:, 0, :], wq_d[:, 0, :])
            emit_xt(13)
            nc.sync.dma_start(wq_sb[:, 1, :], wq_d[:, 1, :])
            emit_xt(14)
            emit_xt(15)
            nc.sync.dma_start(wq_sb[:, 2, :], wq_d[:, 2, :])
            nc.sync.dma_start(wq_sb[:, 3, :], wq_d[:, 3, :])
            nc.sync.dma_start(wo_sb[:], wo_d[:])
            nc.sync.dma_start(tril_sb[:], tril_d[:])

            # ---- Phase 1a: k/v projections, interleaved per chunk; v lags
            # so k's PSUM drains overlap v's tail.
            with tc.tile_pool(name="pp", bufs=2, space="PSUM") as pp:
                LAG = 6
                ps_k = pp.tile([128, T], F32, tag="pp")
                ps_v = pp.tile([128, T], F32, tag="pp")
                for ec in range(ECH + LAG):
                    for which in range(2):
                        e = ec if which == 0 else ec - LAG
                        if which == 0 and e >= ECH:
                            continue
                        if which == 1 and e < 0:
                            continue
                        ps = ps_k if which == 0 else ps_v
                        w_sb = wk_sb if which == 0 else wv_sb
                        for t5 in range(TC):
                            nc.tensor.matmul(
                                ps[:, t5 * 512:(t5 + 1) * 512],
                                w_sb[:, e * D:(e + 1) * D],
                                xt_sb[:, e, t5 * 512:(t5 + 1) * 512],
                                start=(e == 0),
                                stop=(e == ECH - 1),
                            )
                for t5 in range(TC):
                    sl = slice(t5 * 512, (t5 + 1) * 512)
                    drain_proj(ps_k, sl, 0, kT_sb[:, sl], t5)
                for t5 in range(TC):
                    sl = slice(t5 * 512, (t5 + 1) * 512)
                    drain_proj(ps_v, sl, 1, vT_sb[:, sl], t5 + 1)

            # ---- v natural layout via SBUF-to-SBUF DMA (XBAR) transposes ----
            for tt in range(TT):
                nc.sync.dma_start_transpose(
                    v_sb[:, tt, :], vT_sb[:, tt * 128:(tt + 1) * 128]
                )

            # ---- Phase 1b + 2/3 share one scope: q projections run on the
            # O-projection PSUM ring, so there is no pool barrier between the
            # last projection and the first attention matmul.
            with (
                tc.tile_pool(name="eb", bufs=3) as ebp,
                tc.tile_pool(name="ntp", bufs=2) as ntp,
                tc.tile_pool(name="rcp", bufs=2) as rcp,
                tc.tile_pool(name="fsb", bufs=4) as fsb,
                tc.tile_pool(name="ps2", bufs=2, space="PSUM") as ps2,
                tc.tile_pool(name="psr", bufs=2, space="PSUM") as psr,
                tc.tile_pool(name="pso", bufs=2, space="PSUM") as pso,
                tc.tile_pool(name="psf", bufs=2, space="PSUM") as psf,
            ):
                for h in range(HPG):
                    for t5 in range(TC):
                        pq = psf.tile([128, 512], F32, tag="pf", name=f"pq{h}{t5}")
                        for ec in range(ECH):
                            nc.tensor.matmul(
                                pq[:],
                                wq_sb[:, h, ec * D:(ec + 1) * D],
                                xt_sb[:, ec, t5 * 512:(t5 + 1) * 512],
                                start=(ec == 0),
                                stop=(ec == ECH - 1),
                            )
                        drain_proj(
                            pq, slice(0, 512), h + 2,
                            qT_sb[:, h, t5 * 512:(t5 + 1) * 512], h * TC + t5,
                        )
                slots = [(qc, h) for qc in range(TC) for h in range(HPG)]
                NS = len(slots)
                LAGS = 2

                def s_work(E, qc, h, t):
                    """S^T tile t for (qc, h): matmul + exp (+ triangle mask)."""
                    off = 128 * max(0, t - 4 * qc)
                    s2 = ps2.tile([128, 512], F32, tag="s2")
                    nc.tensor.matmul(
                        s2[:, off:512],
                        kT_sb[:, t * D:(t + 1) * D],
                        qT_sb[:, h, qc * 512 + off:(qc + 1) * 512],
                        start=True,
                        stop=True,
                    )
                    nc.scalar.activation(
                        E[:, t, off:512], s2[:, off:512], AF.Exp, scale=SCALE
                    )
                    if t >= 4 * qc:
                        nc.gpsimd.tensor_mul(
                            E[:, t, off:off + 128], E[:, t, off:off + 128], tril_sb[:]
                        )

                def o_proj(nT, qc):
                    """partial out[qc-chunk] = attn^T.T @ Wo_rows, acc over heads."""
                    for qt in range(4):
                        row = qc * 4 + qt
                        for ecol in range(4):
                            pf = psf.tile([128, 512], F32, tag="pf")
                            for h in range(HPG):
                                nc.tensor.matmul(
                                    pf[:],
                                    nT[:, h, qt * 128:(qt + 1) * 128],
                                    wo_sb[:, h * EMBED + ecol * 512:
                                          h * EMBED + (ecol + 1) * 512],
                                    start=(h == 0),
                                    stop=(h == HPG - 1),
                                )
                            f_t = fsb.tile([128, 512], BF, tag="f")
                            nc.vector.tensor_copy(f_t[:], pf[:])
                            nc.sync.dma_start(
                                out_d[row * 128:(row + 1) * 128,
                                      ecol * 512:(ecol + 1) * 512],
                                f_t[:],
                            )

                Es = {}
                nTs = {}
                for i in range(NS + LAGS):
                    if i < NS:
                        qc, h = slots[i]
                        nk = 4 * (qc + 1)
                        Es[i] = ebp.tile([128, TT, 512], BF, tag="E", name=f"E{i}")
                    if i >= LAGS:
                        pqc, ph = slots[i - LAGS]
                        pnk = 4 * (pqc + 1)
                        pE = Es[i - LAGS]
                        po = pso.tile([128, 512], F32, tag="po")
                        sums4 = rcp.tile(
                            [128, 4], BF, tag="sums4", name=f"sums{i}"
                        )
                    nt = nk if i < NS else pnk
                    for t in range(nt):
                        if i < NS:
                            s_work(Es[i], qc, h, t)
                        if i >= LAGS and t < pnk:
                            poff = 128 * max(0, t - 4 * pqc)
                            nc.tensor.matmul(
                                po[:, poff:512], v_sb[:, t, :], pE[:, t, poff:512],
                                start=(t == 0), stop=(t == pnk - 1),
                            )
                        if i >= LAGS and t < 4:
                            # rowsums for q-subtile t: E subtile stationary,
                            # 4-wide ones moving -- near-zero PE occupancy
                            sub = t
                            r_end = 4 * pqc + sub
                            rcol = psr.tile(
                                [128, 512], F32, tag="rs", name=f"rc{i}_{sub}"
                            )
                            for tk in range(r_end + 1):
                                nc.tensor.matmul(
                                    rcol[:, 0:4],
                                    pE[:, tk, sub * 128:(sub + 1) * 128],
                                    ones128[:, 0:4],
                                    start=(tk == 0), stop=(tk == r_end),
                                )
                            nc.vector.tensor_copy(
                                sums4[:, sub:sub + 1], rcol[:, 0:1]
                            )
                    if i >= LAGS:
                        if ph == 0:
                            nTs[pqc] = ntp.tile(
                                [128, HPG, 512], BF, tag="nt", name=f"nT{pqc}"
                            )
                        for sub in range(4):
                            psT = psr.tile(
                                [1, 512], BF, tag="rs", name=f"psT{i}_{sub}"
                            )
                            nc.tensor.transpose(
                                psT[:, 0:128], sums4[:, sub:sub + 1], ident[:]
                            )
                            recipT = rcp.tile(
                                [1, 128], BF, tag="recipT", name=f"rT{i}_{sub}"
                            )
                            with nc.allow_low_precision(reason="softmax recip"):
                                nc.vector.reciprocal(recipT[:], psT[:, 0:128])
                            rb = psr.tile(
                                [128, 512], F32, tag="rs", name=f"rb{i}_{sub}"
                            )
                            nc.tensor.matmul(
                                rb[:, 0:128], ones128[0:1, :], recipT[:],
                                start=True, stop=True,
                            )
                            rbs = rcp.tile(
                                [128, 128], F32, tag="rbs", name=f"rbs{i}_{sub}"
                            )
                            nc.vector.tensor_copy(rbs[:], rb[:, 0:128])
                            nc.vector.tensor_mul(
                                nTs[pqc][:, ph, sub * 128:(sub + 1) * 128],
                                po[:, sub * 128:(sub + 1) * 128], rbs[:],
                            )
                        if ph == HPG - 1:
                            o_proj(nTs[pqc], pqc)

    nc.compile()
    _PROG["nc"] = nc
    return nc


def prepare_in_maps(x, Wq, bq, Wk, bk, Wv, bv, Wo, bo):
    bf = ml_dtypes.bfloat16
    # lower-triangle mask for the 128x128 diagonal tile of S^T:
    # element (p, f) is valid iff k offset p <= q offset f
    p = np.arange(128)[:, None]
    f = np.arange(128)[None, :]
    tril = (f >= p).astype(bf)

    in_maps = []
    for c in range(NCORES):
        b, g = c // 4, c % 4
        xt = x[b].T.astype(bf).reshape(ECH, 128, T)
        # weights partition-major: [128, ec * ...] with contiguous 2KB+ rows
        wq = np.ascontiguousarray(
            Wq[:, g * 512:(g + 1) * 512].astype(bf)
            .reshape(ECH, 128, HPG, D).transpose(1, 2, 0, 3)
            .reshape(128, HPG, ECH * D)
        )
        wk = np.ascontiguousarray(
            Wk[:, g * D:(g + 1) * D].astype(bf)
            .reshape(ECH, 128, D).transpose(1, 0, 2).reshape(128, ECH * D)
        )
        wv = np.ascontiguousarray(
            Wv[:, g * D:(g + 1) * D].astype(bf)
            .reshape(ECH, 128, D).transpose(1, 0, 2).reshape(128, ECH * D)
        )
        wo = np.ascontiguousarray(
            Wo[g * 512:(g + 1) * 512, :].astype(bf)
            .reshape(HPG, 128, EMBED).transpose(1, 0, 2)
            .reshape(128, HPG * EMBED)
        )
        bqc = np.ascontiguousarray(
            bq[g * 512:(g + 1) * 512].reshape(HPG, 128).T
        ).astype(np.float32)
        bkc = bk[g * D:(g + 1) * D].reshape(128, 1).astype(np.float32)
        bvc = bv[g * D:(g + 1) * D].reshape(128, 1).astype(np.float32)
        in_maps.append(
            {
                "xt": xt,
                "wq": wq,
                "wk": wk,
                "wv": wv,
                "wo": wo,
                "tril": tril,
                "bq": bqc,
                "bk": bkc,
                "bv": bvc,
            }
        )
    return in_maps


def combine_outputs(results, bo):
    out = np.empty((2, T, EMBED), dtype=np.float32)
    for b in range(2):
        acc = results[b * 4]["out"].astype(np.float32)
        for g in range(1, 4):
            acc += results[b * 4 + g]["out"].astype(np.float32)
        out[b] = acc + bo[None, :].astype(np.float32)
    return out


def kernel(x, Wq, bq, Wk, bk, Wv, bv, Wo, bo):
    from concourse.bass_utils import run_bass_kernel_spmd

    nc = build_program()
    in_maps = prepare_in_maps(x, Wq, bq, Wk, bk, Wv, bv, Wo, bo)
    res = run_bass_kernel_spmd(nc, in_maps, list(range(NCORES)))
    return combine_outputs(res.results, np.asarray(bo))

